# revision 15
# baseline (speedup 1.0000x reference)
"""HardSort kernel for Trainium2 (Bass/Tile), 8-core data parallel.

Reference semantics (verified bit-exact vs the jax reference):
  For scores [B, n]: output P [B, n, n] with
    rank[c] = #{j : s_j > s_c}
    P[r, c] = 1.0 where rank[c] == r, else 0.0
    rows r where #{c : rank[c] == r} != 1 (duplicated score values) are all-NaN
      (the reference divides an all-zero row by its zero max -> 0/0 = NaN).

Device algorithm per batch row (n=1024, 8 rows per core):
  1. gpsimd partition_broadcast: s_rep[128, 1024] = s row replicated.
  2. rank pass: 8x tensor_scalar(s_rep, is_gt, scalar=s[128k+p], accum_out)
     -> rank_col[p, k] = rank of column 128k+p. Exact f32 integers.
  3. rank -> f16 (exact for ints < 2048), tiny shuffle DMA to [1, 1024] row,
     PE matmul (ones[1,128].T @ rank_row) -> rank_rep[128, 1024] in PSUM.
  4. eq pass: out[p, c] = (rank_rep[p, c] == p + 128k), 8 chunks -> DMA out.
     Split DVE (tensor_scalar is_equal) / ACT (Square + Relu trick).
  5. tie slots only: accum_out of eq pass gives per-row match counts;
     nanadd = (bad*inf) - (bad*inf) in {0, NaN}; out += nanadd per partition.
"""

import numpy as np

N = 1024
NCORES = 8
B = 64
BPC = B // NCORES  # batches per core
NCHUNK = N // 128  # 8

_cache = {}


def _build(tie_slots: frozenset):
    import concourse.bass as bass
    import concourse.tile as tile
    from concourse import bacc, mybir

    f32 = mybir.dt.float32
    f16 = mybir.dt.float16
    bf16 = mybir.dt.bfloat16
    Alu = mybir.AluOpType
    Act = mybir.ActivationFunctionType

    nc = bacc.Bacc(
        "TRN2", target_bir_lowering=False, debug=False, num_devices=NCORES
    )

    scores_rows = nc.dram_tensor(
        "scores_rows", [BPC, N], f32, kind="ExternalInput"
    ).ap()
    scores_cols = nc.dram_tensor(
        "scores_cols", [128, BPC * NCHUNK], f32, kind="ExternalInput"
    ).ap()
    y = nc.dram_tensor("y", [BPC, N, N], f32, kind="ExternalOutput").ap()

    with tile.TileContext(nc) as tc:
        with (
            tc.tile_pool(name="const", bufs=1) as cpool,
            tc.tile_pool(name="srep", bufs=2) as srep_pool,
            tc.tile_pool(name="g", bufs=3) as gpool,
            tc.tile_pool(name="sq", bufs=3) as sqpool,
            tc.tile_pool(name="rank", bufs=2) as rkpool,
            tc.tile_pool(name="out", bufs=6) as opool,
            tc.tile_pool(name="eqt", bufs=NCHUNK + 1) as eqpool,
            tc.tile_pool(name="psum", bufs=2, space=bass.MemorySpace.PSUM) as ppool,
        ):
            # ---- constants ----
            scols = cpool.tile([128, BPC * NCHUNK], f32, tag="scols")
            nc.sync.dma_start(scols[:, :], scores_cols[:, :])
            # iota_tab[p, k] = p + 128k  (f32 exact), neg_iota = -(p + 128k)
            iota_tab = cpool.tile([128, NCHUNK], f32, tag="iota")
            nc.gpsimd.iota(
                iota_tab[:, :], [[128, NCHUNK]], base=0, channel_multiplier=1,
                allow_small_or_imprecise_dtypes=True,
            )
            neg_iota = cpool.tile([128, NCHUNK], f32, tag="niota")
            nc.gpsimd.iota(
                neg_iota[:, :], [[-128, NCHUNK]], base=0, channel_multiplier=-1,
                allow_small_or_imprecise_dtypes=True,
            )
            ones16 = cpool.tile([1, 128], f16, tag="ones16")
            nc.vector.memset(ones16[:, :], 1.0)

            for b in range(BPC):
                is_tie = b in tie_slots
                # ---- 1. replicate scores row across partitions ----
                s_row = srep_pool.tile([1, N], f32, tag="s_row")
                nc.sync.dma_start(s_row[:, :], scores_rows[b : b + 1, :])
                s_rep = srep_pool.tile([128, N], f32, tag="s_rep")
                nc.gpsimd.partition_broadcast(s_rep[:, :], s_row[:, :])

                # ---- 2. rank pass ----
                rank_col = rkpool.tile([128, NCHUNK], f32, tag="rank_col")
                for k in range(NCHUNK):
                    g = gpool.tile([128, N], bf16, tag="g")
                    eng = nc.vector  # Pool has no TensorScalarPtr on HW
                    eng.tensor_scalar(
                        g[:, :],
                        s_rep[:, :],
                        scols[:, b * NCHUNK + k : b * NCHUNK + k + 1],
                        None,
                        Alu.is_gt,
                        Alu.add,
                        accum_out=rank_col[:, k : k + 1],
                    )

                # ---- 3. rank -> [1, 1024] f16 row, PE broadcast to PSUM ----
                rank16 = rkpool.tile([128, NCHUNK], f16, tag="rank16")
                nc.scalar.copy(rank16[:, :], rank_col[:, :])
                rank_row = rkpool.tile([1, N], f16, tag="rank_row")
                # dst[0, 8p + k] = src[p, k]  (column c = 8p + k, partition-major)
                rr_view = rank_row[0:1, :].rearrange("a (p k) -> a p k", p=128)
                nc.sync.dma_start(rr_view, rank16[:, :])
                rank_rep = ppool.tile([128, N], f32, tag="rank_rep")
                for h in range(2):
                    nc.tensor.matmul(
                        rank_rep[:, h * 512 : (h + 1) * 512],
                        ones16[:, :],
                        rank_row[:, h * 512 : (h + 1) * 512],
                        start=True,
                        stop=True,
                    )

                # ---- 4. eq pass (+ row-match counts for tie slots) ----
                if is_tie:
                    rowcnt = rkpool.tile([128, NCHUNK], f32, tag="rowcnt")
                else:
                    rowcnt = None
                out_tiles = []
                eq_tiles = []
                for k in range(NCHUNK):
                    out_t = opool.tile([128, N], f32, tag="out")
                    out_tiles.append(out_t)
                    if is_tie:
                        # eq result lands in scratch; NaN-add writes out_t
                        eq_t = eqpool.tile([128, N], f32, tag="eqt")
                        eq_tiles.append(eq_t)
                        out_t = eq_t
                    if k < 2:
                        # DVE: one fused compare
                        if is_tie:
                            nc.vector.tensor_scalar(
                                out_t[:, :],
                                rank_rep[:, :],
                                iota_tab[:, k : k + 1],
                                None,
                                Alu.is_equal,
                                Alu.add,
                                accum_out=rowcnt[:, k : k + 1],
                            )
                        else:
                            nc.vector.tensor_scalar(
                                out_t[:, :],
                                rank_rep[:, :],
                                iota_tab[:, k : k + 1],
                                None,
                                Alu.is_equal,
                            )
                    else:
                        # ACT: sq = (rank - (p + 128k))^2 ; out = relu(1 - sq)
                        sq = sqpool.tile([128, N], bf16, tag="sq")
                        nc.scalar.activation(
                            sq[:, :],
                            rank_rep[:, :],
                            Act.Square,
                            bias=neg_iota[:, k : k + 1],
                            scale=1.0,
                        )
                        if is_tie:
                            nc.scalar.activation(
                                out_t[:, :],
                                sq[:, :],
                                Act.Relu,
                                bias=1.0,
                                scale=-1.0,
                                accum_out=rowcnt[:, k : k + 1],
                            )
                        else:
                            nc.scalar.activation(
                                out_t[:, :], sq[:, :], Act.Relu, bias=1.0, scale=-1.0
                            )

                # ---- 5. NaN patch for tie slots ----
                if is_tie:
                    bad = rkpool.tile([128, NCHUNK], f32, tag="bad")
                    nc.vector.tensor_scalar(
                        bad[:, :], rowcnt[:, :], 1.0, None, Alu.not_equal
                    )
                    # (bad * 3e38) * 3e38 -> {0, inf} (no 0*inf NaN trap)
                    badinf = rkpool.tile([128, NCHUNK], f32, tag="badinf")
                    nc.vector.tensor_scalar(
                        badinf[:, :], bad[:, :], 3.0e38, 3.0e38, Alu.mult, Alu.mult
                    )
                    nanadd = rkpool.tile([128, NCHUNK], f32, tag="nanadd")
                    nc.vector.tensor_tensor(
                        nanadd[:, :], badinf[:, :], badinf[:, :], Alu.subtract
                    )
                    for k in range(NCHUNK):
                        nc.vector.tensor_scalar(
                            out_tiles[k][:, :],
                            eq_tiles[k][:, :],
                            nanadd[:, k : k + 1],
                            None,
                            Alu.add,
                        )

                # ---- 6. DMA out ----
                for k in range(NCHUNK):
                    nc.sync.dma_start(
                        y[b, k * 128 : (k + 1) * 128, :], out_tiles[k][:, :]
                    )

    nc.compile()
    return nc


def _prep_core_inputs(rows: np.ndarray):
    # rows: [BPC, N] f32 for one core
    # scores_cols[p, b*NCHUNK + k] = rows[b, NCHUNK*p + k]  (column c = 8p + k)
    cols = np.ascontiguousarray(
        rows.reshape(BPC, 128, NCHUNK).transpose(1, 0, 2).reshape(128, BPC * NCHUNK)
    )
    return {"scores_rows": np.ascontiguousarray(rows), "scores_cols": cols}


LAST_EXEC_NS = None


def kernel(scores: np.ndarray) -> np.ndarray:
    global LAST_EXEC_NS
    import os
    from concourse import bass_utils

    scores = np.ascontiguousarray(np.asarray(scores, dtype=np.float32))
    assert scores.shape == (B, N), scores.shape
    shards = scores.reshape(NCORES, BPC, N)

    # Tie slots: union across cores so the single SPMD program fits all cores.
    tie = set()
    for c in range(NCORES):
        for s in range(BPC):
            if np.unique(shards[c, s]).size != N:
                tie.add(s)
    tie = frozenset(tie)

    if tie not in _cache:
        _cache[tie] = _build(tie)
    nc = _cache[tie]

    in_maps = [_prep_core_inputs(shards[c]) for c in range(NCORES)]
    trace = bool(os.environ.get("HS_TRACE"))
    res = bass_utils.run_bass_kernel_spmd(
        nc, in_maps, core_ids=list(range(NCORES)), trace=trace
    )
    LAST_EXEC_NS = res.exec_time_ns
    out = np.concatenate([res.results[c]["y"] for c in range(NCORES)], axis=0)
    return out


# revision 17
# speedup vs baseline: 1.1148x; 1.1148x over previous
"""HardSort kernel for Trainium2 (Bass/Tile), 8-core data parallel.

Reference semantics (verified bit-exact vs the jax reference):
  For scores [B, n]: output P [B, n, n] with
    rank[c] = #{j : s_j > s_c}
    P[r, c] = 1.0 where rank[c] == r, else 0.0
    rows r where #{c : rank[c] == r} != 1 (duplicated score values) are all-NaN
      (the reference divides an all-zero row by its zero max -> 0/0 = NaN).

Device algorithm per batch row (n=1024, 8 rows per core):
  1. gpsimd partition_broadcast: s_rep[128, 1024] = s row replicated.
  2. rank pass: 8x tensor_scalar(s_rep, is_gt, scalar=s[128k+p], accum_out)
     -> rank_col[p, k] = rank of column 128k+p. Exact f32 integers.
  3. rank -> f16 (exact for ints < 2048), tiny shuffle DMA to [1, 1024] row,
     PE matmul (ones[1,128].T @ rank_row) -> rank_rep[128, 1024] in PSUM.
  4. eq pass: out[p, c] = (rank_rep[p, c] == p + 128k), 8 chunks -> DMA out.
     Split DVE (tensor_scalar is_equal) / ACT (Square + Relu trick).
  5. tie slots only: accum_out of eq pass gives per-row match counts;
     nanadd = (bad*inf) - (bad*inf) in {0, NaN}; out += nanadd per partition.
"""

import numpy as np

N = 1024
NCORES = 8
B = 64
BPC = B // NCORES  # batches per core
NCHUNK = N // 128  # 8

_cache = {}


def _build(tie_slots: frozenset):
    import concourse.bass as bass
    import concourse.tile as tile
    from concourse import bacc, mybir

    f32 = mybir.dt.float32
    f16 = mybir.dt.float16
    bf16 = mybir.dt.bfloat16
    Alu = mybir.AluOpType
    Act = mybir.ActivationFunctionType

    nc = bacc.Bacc(
        "TRN2", target_bir_lowering=False, debug=False, num_devices=NCORES
    )

    scores_rows = nc.dram_tensor(
        "scores_rows", [BPC, N], f32, kind="ExternalInput"
    ).ap()
    scores_cols = nc.dram_tensor(
        "scores_cols", [128, BPC * NCHUNK], f32, kind="ExternalInput"
    ).ap()
    y = nc.dram_tensor("y", [BPC, N, N], f32, kind="ExternalOutput").ap()

    with tile.TileContext(nc) as tc:
        with (
            tc.tile_pool(name="const", bufs=1) as cpool,
            tc.tile_pool(name="srep", bufs=2) as srep_pool,
            tc.tile_pool(name="g", bufs=3) as gpool,
            tc.tile_pool(name="sq", bufs=3) as sqpool,
            tc.tile_pool(name="rank", bufs=2) as rkpool,
            tc.tile_pool(name="out", bufs=6) as opool,
            tc.tile_pool(name="eqt", bufs=NCHUNK + 1) as eqpool,
            tc.tile_pool(name="psum", bufs=2, space=bass.MemorySpace.PSUM) as ppool,
        ):
            # ---- constants ----
            scols = cpool.tile([128, BPC * NCHUNK], f32, tag="scols")
            nc.sync.dma_start(scols[:, :], scores_cols[:, :])
            # iota_tab[p, k] = p + 128k  (f32 exact), neg_iota = -(p + 128k)
            iota_tab = cpool.tile([128, NCHUNK], f32, tag="iota")
            nc.gpsimd.iota(
                iota_tab[:, :], [[128, NCHUNK]], base=0, channel_multiplier=1,
                allow_small_or_imprecise_dtypes=True,
            )
            neg_iota = cpool.tile([128, NCHUNK], f32, tag="niota")
            nc.gpsimd.iota(
                neg_iota[:, :], [[-128, NCHUNK]], base=0, channel_multiplier=-1,
                allow_small_or_imprecise_dtypes=True,
            )
            ones16 = cpool.tile([1, 128], f16, tag="ones16")
            nc.vector.memset(ones16[:, :], 1.0)
            ones32 = cpool.tile([1, 128], f32, tag="ones32")
            nc.vector.memset(ones32[:, :], 1.0)

            for b in range(BPC):
                is_tie = b in tie_slots
                # ---- 1. replicate scores row across partitions (PE, exact) ----
                s_row = srep_pool.tile([1, N], f32, tag="s_row")
                nc.sync.dma_start(s_row[:, :], scores_rows[b : b + 1, :])
                s_rep = ppool.tile([128, N], f32, tag="s_rep")
                for h in range(2):
                    nc.tensor.matmul(
                        s_rep[:, h * 512 : (h + 1) * 512],
                        ones32[:, :],
                        s_row[:, h * 512 : (h + 1) * 512],
                        start=True,
                        stop=True,
                    )

                # ---- 2. rank pass ----
                rank_col = rkpool.tile([128, NCHUNK], f32, tag="rank_col")
                for k in range(NCHUNK):
                    g = gpool.tile([128, N], bf16, tag="g")
                    eng = nc.vector  # Pool has no TensorScalarPtr on HW
                    eng.tensor_scalar(
                        g[:, :],
                        s_rep[:, :],
                        scols[:, b * NCHUNK + k : b * NCHUNK + k + 1],
                        None,
                        Alu.is_gt,
                        Alu.add,
                        accum_out=rank_col[:, k : k + 1],
                    )

                # ---- 3. rank -> [1, 1024] f16 row, PE broadcast to PSUM ----
                rank16 = rkpool.tile([128, NCHUNK], f16, tag="rank16")
                nc.scalar.copy(rank16[:, :], rank_col[:, :])
                rank_row = rkpool.tile([1, N], f16, tag="rank_row")
                # dst[0, 8p + k] = src[p, k]  (column c = 8p + k, partition-major)
                rr_view = rank_row[0:1, :].rearrange("a (p k) -> a p k", p=128)
                nc.sync.dma_start(rr_view, rank16[:, :])
                rank_rep = ppool.tile([128, N], f32, tag="rank_rep")
                for h in range(2):
                    nc.tensor.matmul(
                        rank_rep[:, h * 512 : (h + 1) * 512],
                        ones16[:, :],
                        rank_row[:, h * 512 : (h + 1) * 512],
                        start=True,
                        stop=True,
                    )

                # ---- 4. eq pass (+ row-match counts for tie slots) ----
                if is_tie:
                    rowcnt = rkpool.tile([128, NCHUNK], f32, tag="rowcnt")
                else:
                    rowcnt = None
                out_tiles = []
                eq_tiles = []
                for k in range(NCHUNK):
                    out_t = opool.tile([128, N], f32, tag="out")
                    out_tiles.append(out_t)
                    if is_tie:
                        # eq result lands in scratch; NaN-add writes out_t
                        eq_t = eqpool.tile([128, N], f32, tag="eqt")
                        eq_tiles.append(eq_t)
                        out_t = eq_t
                    if k < 2:
                        # DVE: one fused compare
                        if is_tie:
                            nc.vector.tensor_scalar(
                                out_t[:, :],
                                rank_rep[:, :],
                                iota_tab[:, k : k + 1],
                                None,
                                Alu.is_equal,
                                Alu.add,
                                accum_out=rowcnt[:, k : k + 1],
                            )
                        else:
                            nc.vector.tensor_scalar(
                                out_t[:, :],
                                rank_rep[:, :],
                                iota_tab[:, k : k + 1],
                                None,
                                Alu.is_equal,
                            )
                    else:
                        # ACT: sq = (rank - (p + 128k))^2 ; out = relu(1 - sq)
                        sq = sqpool.tile([128, N], bf16, tag="sq")
                        nc.scalar.activation(
                            sq[:, :],
                            rank_rep[:, :],
                            Act.Square,
                            bias=neg_iota[:, k : k + 1],
                            scale=1.0,
                        )
                        if is_tie:
                            nc.scalar.activation(
                                out_t[:, :],
                                sq[:, :],
                                Act.Relu,
                                bias=1.0,
                                scale=-1.0,
                                accum_out=rowcnt[:, k : k + 1],
                            )
                        else:
                            nc.scalar.activation(
                                out_t[:, :], sq[:, :], Act.Relu, bias=1.0, scale=-1.0
                            )

                # ---- 5. NaN patch for tie slots ----
                if is_tie:
                    bad = rkpool.tile([128, NCHUNK], f32, tag="bad")
                    nc.vector.tensor_scalar(
                        bad[:, :], rowcnt[:, :], 1.0, None, Alu.not_equal
                    )
                    # (bad * 3e38) * 3e38 -> {0, inf} (no 0*inf NaN trap)
                    badinf = rkpool.tile([128, NCHUNK], f32, tag="badinf")
                    nc.vector.tensor_scalar(
                        badinf[:, :], bad[:, :], 3.0e38, 3.0e38, Alu.mult, Alu.mult
                    )
                    nanadd = rkpool.tile([128, NCHUNK], f32, tag="nanadd")
                    nc.vector.tensor_tensor(
                        nanadd[:, :], badinf[:, :], badinf[:, :], Alu.subtract
                    )
                    for k in range(NCHUNK):
                        nc.vector.tensor_scalar(
                            out_tiles[k][:, :],
                            eq_tiles[k][:, :],
                            nanadd[:, k : k + 1],
                            None,
                            Alu.add,
                        )

                # ---- 6. DMA out ----
                for k in range(NCHUNK):
                    nc.sync.dma_start(
                        y[b, k * 128 : (k + 1) * 128, :], out_tiles[k][:, :]
                    )

    nc.compile()
    return nc


def _prep_core_inputs(rows: np.ndarray):
    # rows: [BPC, N] f32 for one core
    # scores_cols[p, b*NCHUNK + k] = rows[b, NCHUNK*p + k]  (column c = 8p + k)
    cols = np.ascontiguousarray(
        rows.reshape(BPC, 128, NCHUNK).transpose(1, 0, 2).reshape(128, BPC * NCHUNK)
    )
    return {"scores_rows": np.ascontiguousarray(rows), "scores_cols": cols}


LAST_EXEC_NS = None


def kernel(scores: np.ndarray) -> np.ndarray:
    global LAST_EXEC_NS
    import os
    from concourse import bass_utils

    scores = np.ascontiguousarray(np.asarray(scores, dtype=np.float32))
    assert scores.shape == (B, N), scores.shape
    shards = scores.reshape(NCORES, BPC, N)

    # Tie slots: union across cores so the single SPMD program fits all cores.
    tie = set()
    for c in range(NCORES):
        for s in range(BPC):
            if np.unique(shards[c, s]).size != N:
                tie.add(s)
    tie = frozenset(tie)

    if tie not in _cache:
        _cache[tie] = _build(tie)
    nc = _cache[tie]

    in_maps = [_prep_core_inputs(shards[c]) for c in range(NCORES)]
    trace = bool(os.environ.get("HS_TRACE"))
    res = bass_utils.run_bass_kernel_spmd(
        nc, in_maps, core_ids=list(range(NCORES)), trace=trace
    )
    LAST_EXEC_NS = res.exec_time_ns
    out = np.concatenate([res.results[c]["y"] for c in range(NCORES)], axis=0)
    return out
